# revision 1
# baseline (speedup 1.0000x reference)
"""Trainium2 Bass kernel for the digit-conv model, v2 (tile-packed).

Math: y = relu(relu(conv3x3(x) @ W1 + b1) @ W2 + b2) @ W3 + b3.
The valid 3x3 conv folds into W1 on device (W1eff = A @ W1 with banded
A^T, computed once on the tensor engine), so the stream is a 3-layer
MLP with channels on partitions and batch on the free dimension.

v2 recovers the ragged-tile waste of v1 (which ran 25 N=512 passes per
512-batch block) with PE array tiling:
  - L1 k-tiles of 128 (6 full + K=16 tail), m-chunks {128, 128, 44}.
    The 12 (m0,m1)x(6 k) passes use the full 128x128 array.
  - The K=16 tail for m0+m1 runs as one 32x128-mode row-tiled span
    (2 concurrent MMs on row strips; even blocks use strips 0,1, odd
    blocks strips 2,3 so a block pair's 4 tail MMs form one span).
    The tail x rows are DMA-replicated to partition groups 0/32/64/96.
  - The M=44 chunk (mt) runs col-paired in 128x64 mode: k-tiles
    {0,2,4,6} accumulate at PSUM parts 0:44 (T0) while {1,3,5} run
    concurrently at parts 64:108 (T1); a cross-partition DVE add
    combines the halves before the relu. The 7th k-tile rides in pa's
    chain as a zero-padded K=128 MM (w1eff6 rows 16:128 are zero).
  - L3 (K=100, M=10) col-packs 4 blocks into one 128x32-mode span
    (tile_position (0,32j), PSUM partition slices 32j:32j+10).
Per 512-block: 17 L1 spans + 3 L2 passes + 0.25 L3 spans vs 25.
Same-mode work is grouped per block pair (S1 tail span, S2 full
passes + L2, S3 mt spans, L3 every other pair) to amortize tiling
mode-switch drains.

All matmul operands bf16 (fp8 DoubleRow fails the 2e-2 gate: measured
4.5e-2), PSUM fp32, biases applied from PSUM in fp32. Fold DMA goes on
the scalar HWDGE queue, x supers + tail replicas on sync (tails first
so the S1 spans don't wait on the big x transfer), small consts and y
outputs on gpsimd SWDGE. HAM warmup + post-fold filler as in v1.
"""

import ml_dtypes
import numpy as np

import concourse.tile as tile
from concourse import bacc, mybir
from concourse import bass_utils

N_CORES = 8
B = 65536
BC = B // N_CORES  # 8192 rows per core
U = 784            # input features (28*28)
Q = 676            # conv outputs (26*26)
QP = 768           # q padded to 6 tiles of 128
H1, H2, H3 = 300, 100, 10
NB = 512           # batch columns per block (one PSUM bank of fp32)
KT = 128           # u-dim k-tile
NKT = 6            # full k-tiles
KTAIL = U - NKT * KT   # 16
MT = 44            # ragged m-chunk width (300 - 2*128)
MTH = 22           # mt half-chunk (one 128x32 col tile per half)
ABW = 384          # banded A^T width: 3 chunks of 128
FW = ABW + H1      # 684 packed fold row width
SUP = 1536         # max batch columns per DMA super-block
SUP_WIDTHS = [512, 512, 1536, 1536, 1536, 1536, 1024]
# first four blocks are 256 wide (with split x DMAs) so compute starts
# earlier on the HBM-bound ramp; the rest are full 512 blocks.
_BLOCK_NB = [256, 256, 256, 256] + [512] * 14
NBLK = 18
assert sum(SUP_WIDTHS) == BC and sum(_BLOCK_NB) == BC

_SUP_START = [0]
for _w in SUP_WIDTHS:
    _SUP_START.append(_SUP_START[-1] + _w)
_BLK2SUP = []  # block -> (super, col offset within super, width)
_bi = 0
for _s, _w in enumerate(SUP_WIDTHS):
    _off = 0
    while _off < _w:
        _nb = _BLOCK_NB[_bi]
        _BLK2SUP.append((_s, _off, _nb))
        _off += _nb
        _bi += 1
    assert _off == _w
assert len(_BLK2SUP) == NBLK
_L3_GROUPS = [[0, 1, 2, 3], [4, 5, 6, 7], [8, 9, 10, 11],
              [12, 13, 14, 15], [16, 17]]

_prog_cache = {}


def _fold_bands():
    """Static block-sparsity of A^T [Q, U]: per 128-row q-tile the nonzero
    columns lie in a band; band given in whole 128-wide u-chunks."""
    bands = []
    for t in range(6):
        q0 = t * 128
        p_real = min(128, Q - q0)
        i_lo = q0 // 26
        i_hi = (q0 + p_real - 1) // 26
        u_lo = 28 * i_lo
        u_hi = min(U, 28 * (i_hi + 3))
        c_lo = u_lo // 128
        c_hi = -(-u_hi // 128)
        assert c_hi - c_lo <= 3
        bands.append((q0, p_real, c_lo, c_hi))
    return bands


def _build_program():
    f32 = mybir.dt.float32
    bf16 = mybir.dt.bfloat16
    relu = mybir.ActivationFunctionType.Relu
    alu_add = mybir.AluOpType.add
    alu_max = mybir.AluOpType.max

    nc = bacc.Bacc(
        "TRN2", target_bir_lowering=False, debug=False, num_devices=N_CORES
    )

    xT_d = nc.dram_tensor("xT", [U, BC], bf16, kind="ExternalInput").ap()
    fold_d = nc.dram_tensor("fold", [QP, FW], bf16, kind="ExternalInput").ap()
    w2_d = nc.dram_tensor("w2", [3 * KT, H2], bf16, kind="ExternalInput").ap()
    w3_d = nc.dram_tensor("w3", [H2, H3], bf16, kind="ExternalInput").ap()
    bias_d = nc.dram_tensor("bias", [128, 5], f32, kind="ExternalInput").ap()
    yT_d = nc.dram_tensor("yT", [H3, BC], f32, kind="ExternalOutput").ap()

    bands = _fold_bands()
    cover = [[t for t, (_, _, cl, ch) in enumerate(bands) if cl <= ut < ch]
             for ut in range(7)]

    with tile.TileContext(nc) as tc:
        with tc.tile_pool(name="const", bufs=1) as cpool, \
             tc.tile_pool(name="xp", bufs=5) as xpool, \
             tc.tile_pool(name="hp", bufs=4) as hpool, \
             tc.tile_pool(name="h2p", bufs=5) as h2pool, \
             tc.tile_pool(name="yp", bufs=2) as ypool, \
             tc.tile_pool(name="ps1", bufs=5, space="PSUM") as ps1p, \
             tc.tile_pool(name="psmt", bufs=1, space="PSUM") as psmtp, \
             tc.tile_pool(name="ps2", bufs=2, space="PSUM") as ps2p:

            # ---- HAM warmup ----
            warm_sb = cpool.tile([128, 512], bf16)
            nc.vector.memset(warm_sb[:], 0.0)
            for wi in range(8):
                pw = psmtp.tile([128, NB], f32, tag="mt", name=f"pwarm_{wi}")
                nc.tensor.matmul(pw[:], warm_sb[:, :128], warm_sb[:],
                                 start=True, stop=True)

            # ---- constants ----
            # fold goes FIRST on the sync queue: the x supers behind it
            # share HBM bandwidth, and everything waits on w1eff.
            fold_sb = cpool.tile([128, 6 * FW], bf16)
            nc.sync.dma_start(
                fold_sb[:].rearrange("p (q c) -> p q c", c=FW),
                fold_d.rearrange("(q p) c -> p q c", p=128),
            )
            w2_sb = cpool.tile([128, 3 * H2], bf16)
            nc.gpsimd.dma_start(
                w2_sb[:].rearrange("p (k c) -> p k c", c=H2),
                w2_d.rearrange("(k p) c -> p k c", p=128),
            )
            w3_sb = cpool.tile([H2, H3], bf16)
            nc.gpsimd.dma_start(w3_sb[:], w3_d)
            bias_sb = cpool.tile([128, 5], f32)
            nc.gpsimd.dma_start(bias_sb[:], bias_d)

            # ---- fold the conv into W1: W1eff[u, c] = (A^T).T @ W1 ----
            w1eff_sb = cpool.tile([128, NKT * H1], bf16)
            w1eff6_sb = cpool.tile([128, H1], bf16)
            nc.vector.memset(w1eff6_sb[:], 0.0)
            fv = fold_sb[:].rearrange("p (q c) -> p q c", c=FW)
            for ut in range(NKT):
                pf = ps1p.tile([128, H1], f32, tag="l1", name=f"pfold_{ut}",
                               padded_shape=[128, NB])
                parts = cover[ut]
                for idx, t in enumerate(parts):
                    c_lo = bands[t][2]
                    off = (ut - c_lo) * 128
                    nc.tensor.matmul(
                        pf[:], fv[:, t, off:off + 128], fv[:, t, ABW:FW],
                        start=(idx == 0), stop=(idx == len(parts) - 1))
                nc.vector.tensor_copy(
                    w1eff_sb[:, ut * H1:(ut + 1) * H1], pf[:])
            # tail chunk (K=16): produce the tail weights at partition
            # groups 0/32/64/96 via col-tiled fold MMs (no cross-partition
            # copies needed): parts 0:16 all 300 cols (m0 even + mt),
            # 32:48 m1-even cols, 64:80 m0-odd cols, 96:112 m1-odd cols.
            t5 = cover[6][0]
            c_lo5 = bands[t5][2]
            lhs6 = fv[:, t5, (6 - c_lo5) * 128:(6 - c_lo5) * 128 + KTAIL]
            pf6 = ps1p.tile([128, H1], f32, tag="l1", name="pfold_6",
                            padded_shape=[128, NB])
            nc.tensor.matmul(pf6[0:KTAIL, :], lhs6, fv[:, t5, ABW:FW],
                             start=True, stop=True)
            nc.tensor.matmul(pf6[32:32 + KTAIL, 0:128], lhs6,
                             fv[:, t5, ABW + 128:ABW + 256],
                             start=True, stop=True, tile_position=(0, 32))
            nc.tensor.matmul(pf6[64:64 + KTAIL, 0:128], lhs6,
                             fv[:, t5, ABW:ABW + 128],
                             start=True, stop=True, tile_position=(0, 64))
            nc.tensor.matmul(pf6[96:96 + KTAIL, 0:128], lhs6,
                             fv[:, t5, ABW + 128:ABW + 256],
                             start=True, stop=True, tile_position=(0, 96))
            nc.vector.tensor_copy(w1eff6_sb[0:KTAIL, :], pf6[0:KTAIL, :])
            nc.vector.tensor_copy(w1eff6_sb[32:32 + KTAIL, 128:256],
                                  pf6[32:32 + KTAIL, 0:128])
            nc.vector.tensor_copy(w1eff6_sb[64:64 + KTAIL, 0:128],
                                  pf6[64:64 + KTAIL, 0:128])
            nc.vector.tensor_copy(w1eff6_sb[96:96 + KTAIL, 128:256],
                                  pf6[96:96 + KTAIL, 0:128])

            # ---- post-fold filler ----
            for wi in range(4):
                pw = psmtp.tile([128, NB], f32, tag="mt", name=f"pfill_{wi}")
                nc.tensor.matmul(pw[:], warm_sb[:, :128], warm_sb[:],
                                 start=True, stop=True)

            w1v = w1eff_sb[:].rearrange("p (k c) -> p k c", c=H1)
            w2v = w2_sb[:].rearrange("p (k c) -> p k c", c=H2)

            xviews = [None] * len(SUP_WIDTHS)
            emitted = [-1]
            psA, psB, pmt, h1t, h2t = {}, {}, {}, {}, {}

            def emit_super(s):
                xt = xpool.tile([128, 7 * SUP], bf16, tag="x", name=f"xt_{s}")
                xv = xt[:].rearrange("p (k c) -> p k c", c=SUP)
                xviews[s] = xv
                # garbage zones of the tail k-tile must be zero: the
                # zero-padded mt tail MM streams all 128 partitions. Done
                # per super so every logical tile has the region written.
                # (DVE partition starts must be 32-aligned, so clear the
                # whole k-tile; the tail DMAs below overwrite their rows.)
                nc.vector.memset(xv[:, 6, :], 0.0)
                sw = SUP_WIDTHS[s]
                c0 = _SUP_START[s]
                for g in range(4):
                    nc.sync.dma_start(
                        xv[32 * g:32 * g + KTAIL, 6, :sw],
                        xT_d[NKT * KT:U, c0:c0 + sw])
                # the first supers land in block-sized pieces so each block
                # starts as soon as its own slice is in (the ramp is
                # HBM-bound; later supers arrive well ahead of use).
                pw = 256 if s < 2 else (512 if s == 2 else sw)
                for o in range(0, sw, pw):
                    nc.sync.dma_start(
                        xv[:, 0:6, o:o + pw],
                        xT_d[0:NKT * KT, c0 + o:c0 + o + pw]
                        .rearrange("(k p) c -> p k c", p=128))

            def ensure_supers(upto_block):
                s_needed = _BLK2SUP[min(upto_block, NBLK - 1)][0]
                while emitted[0] < s_needed:
                    emitted[0] += 1
                    emit_super(emitted[0])

            def xview(b):
                s, off, nb = _BLK2SUP[b]
                return xviews[s], off, nb

            def alloc_h1(b):
                if b in h1t:
                    return
                h1 = hpool.tile([128, 3 * NB], bf16, tag="h1", name=f"h1_{b}")
                h1t[b] = h1
                h1v = h1[:].rearrange("p (k c) -> p k c", c=NB)
                # parts outside the mt halves in the third chunk must be
                # zero for the zero-padded L2 k3 pass; cleared per block so
                # every logical tile has the region written (the mt relus
                # overwrite their slices).
                nc.vector.memset(h1v[:, 2, :], 0.0)

            def s1_block(b):
                # k-tail span: m0+m1 tails row-tiled (opens the psum groups)
                xv, off, nb = xview(b)
                pA = ps1p.tile([128, NB], f32, tag="l1", name=f"pA_{b}")
                pB = ps1p.tile([128, NB], f32, tag="l1", name=f"pB_{b}")
                psA[b], psB[b] = pA, pB
                alloc_h1(b)
                p0 = 0 if b % 2 == 0 else 64
                p1 = p0 + 32
                nc.tensor.matmul(pA[:, :nb], w1eff6_sb[p0:p0 + KTAIL, 0:128],
                                 xv[p0:p0 + KTAIL, 6, off:off + nb],
                                 start=True, stop=False)
                kw = {"tile_position": (96, 0)} if p1 == 96 else {}
                nc.tensor.matmul(pB[:, :nb], w1eff6_sb[p1:p1 + KTAIL, 128:256],
                                 xv[p1:p1 + KTAIL, 6, off:off + nb],
                                 start=True, stop=False, **kw)

            def s2_block(b):
                # 12 full 128x128 passes + relu drains for m0/m1
                xv, off, nb = xview(b)
                pA, pB = psA[b], psB[b]
                h1v = h1t[b][:].rearrange("p (k c) -> p k c", c=NB)
                for kt in range(NKT):
                    nc.tensor.matmul(pA[:, :nb], w1v[:, kt, 0:128],
                                     xv[:, kt, off:off + nb],
                                     start=False, stop=(kt == NKT - 1))
                nc.scalar.activation(h1v[:, 0, :nb], pA[:, :nb], relu,
                                     bias=bias_sb[:, 0:1], scale=1.0)
                for kt in range(NKT):
                    nc.tensor.matmul(pB[:, :nb], w1v[:, kt, 128:256],
                                     xv[:, kt, off:off + nb],
                                     start=False, stop=(kt == NKT - 1))
                nc.scalar.activation(h1v[:, 1, :nb], pB[:, :nb], relu,
                                     bias=bias_sb[:, 1:2], scale=1.0)

            def s3_pair(b0, b1):
                # mt (44 channels) split 22+22 across col tiles: 4 chains
                # (2 halves x 2 blocks) run concurrently in 128x32 mode at
                # PSUM partition slices 0:22 / 32:54 / 64:86 / 96:118, each
                # contracting all 7 k-tiles. The four chains are independent
                # per-partition accumulation groups in one bank; the bass
                # group checker only models bank-granular groups, so skip it.
                pm = psmtp.tile([128, NB], f32, tag="mt", name=f"pmt_{b0}")
                alloc_h1(b0)
                alloc_h1(b1)
                chains = []  # (psum base, block, w1 col lo/hi)
                for j, b in enumerate((b0, b0, b1, b1)):
                    lo = 256 + (j % 2) * MTH
                    chains.append((32 * j, b, lo, lo + MTH))
                for kt in range(NKT + 1):
                    for (pb_, b, lo, hi) in chains:
                        xv, off, nb = xview(b)
                        if kt < NKT:
                            lhs = w1v[:, kt, lo:hi]
                        else:
                            lhs = w1eff6_sb[:, lo:hi]
                        nc.tensor.matmul(pm[pb_:pb_ + MTH, :nb], lhs,
                                         xv[:, min(kt, 6), off:off + nb],
                                         start=(kt == 0), stop=(kt == NKT),
                                         skip_group_check=True,
                                         tile_position=(0, pb_))
                for (pb_, b, lo, hi) in chains:
                    nb = _BLK2SUP[b][2]
                    h1v = h1t[b][:].rearrange("p (k c) -> p k c", c=NB)
                    nc.scalar.activation(
                        h1v[pb_:pb_ + MTH, 2, :nb], pm[pb_:pb_ + MTH, :nb],
                        relu, bias=bias_sb[pb_:pb_ + MTH, 2:3], scale=1.0)

            p2t = {}

            def l2_head(b):
                # the two full K=128 passes of L2 (group opened here)
                nb = _BLK2SUP[b][2]
                h1v = h1t[b][:].rearrange("p (k c) -> p k c", c=NB)
                p2 = ps2p.tile([H2, NB], f32, tag="l2", name=f"p2_{b}",
                               padded_shape=[128, NB])
                p2t[b] = p2
                for j in range(2):
                    nc.tensor.matmul(p2[:, :nb], w2v[:, j, :],
                                     h1v[:, j, :nb],
                                     start=(j == 0), stop=False)

            def l2_tail_pair(b0, b1):
                # k3 (real K=44, zero-padded): the mt halves of even blocks
                # live at parts 0:54, odd at 64:118, so an even+odd pair of
                # k3 passes row-tiles into one 64x128-mode span (different
                # psum banks), then both h2 relus drain.
                for i, b in enumerate((b0, b1)):
                    base = 64 * (b % 2)
                    nb = _BLK2SUP[b][2]
                    h1v = h1t[b][:].rearrange("p (k c) -> p k c", c=NB)
                    nc.tensor.matmul(p2t[b][:, :nb],
                                     w2v[base:base + 64, 2, :],
                                     h1v[base:base + 64, 2, :nb],
                                     start=False, stop=True,
                                     tile_position=(base, 0))
                for b in (b0, b1):
                    nb = _BLK2SUP[b][2]
                    h2 = h2pool.tile([H2, NB], bf16, tag="h2", name=f"h2_{b}")
                    h2t[b] = h2
                    nc.vector.tensor_scalar(h2[:, :nb], p2t[b][:, :nb],
                                            bias_sb[0:H2, 3:4], 0.0,
                                            alu_add, alu_max)

            def l3_span(bs, j0):
                # blocks col-packed in 128x32 mode, then bias + store
                p3 = ps2p.tile([128, NB], f32, tag="l2", name=f"p3_{bs[0]}")
                y = ypool.tile([128, NB], f32, tag="y", name=f"y_{bs[0]}")
                for i, b in enumerate(bs):
                    j = j0 + i
                    nb = _BLK2SUP[b][2]
                    nc.tensor.matmul(p3[32 * j:32 * j + H3, :nb], w3_sb[:],
                                     h2t[b][:, :nb], start=True, stop=True,
                                     tile_position=(0, 32 * j))
                for i, b in enumerate(bs):
                    j = j0 + i
                    s, off, nb = _BLK2SUP[b]
                    c0 = _SUP_START[s] + off
                    nc.vector.tensor_scalar_add(
                        y[32 * j:32 * j + H3, :nb],
                        p3[32 * j:32 * j + H3, :nb],
                        bias_sb[32 * j:32 * j + H3, 4:5])
                    nc.gpsimd.dma_start(yT_d[:, c0:c0 + nb],
                                        y[32 * j:32 * j + H3, :nb])

            next_g = [0]

            def fire_l3(upto_block):
                while (next_g[0] < len(_L3_GROUPS) - 1
                       and _L3_GROUPS[next_g[0]][-1] <= upto_block):
                    l3_span(_L3_GROUPS[next_g[0]], 0)
                    next_g[0] += 1

            for p in range(NBLK // 2):
                b0, b1 = 2 * p, 2 * p + 1
                ensure_supers(b1 + 4)
                last = p == NBLK // 2 - 1
                s1_block(b0)
                s1_block(b1)
                if last:
                    # last pair: mt before the m-chains so its relus are
                    # long done when the trailing L2/L3 chain reads them.
                    s3_pair(b0, b1)
                s2_block(b0)
                s2_block(b1)
                # L2 of the previous pair goes after this pair's m-chains so
                # the relus that feed it (end of previous pair) are long
                # done by the time its passes read h1.
                if p >= 1:
                    l2_head(b0 - 2)
                    l2_head(b1 - 2)
                    l2_tail_pair(b0 - 2, b1 - 2)
                if not last:
                    s3_pair(b0, b1)
                if p >= 1:
                    fire_l3(b1 - 2)
            # epilogue: the last pair's L2s, then the final L3 group in two
            # spans so only one block's bias-add + store trails the stream.
            l2_head(NBLK - 2)
            l2_head(NBLK - 1)
            l2_tail_pair(NBLK - 2, NBLK - 1)
            l3_span(_L3_GROUPS[-1][:-1], 0)
            l3_span(_L3_GROUPS[-1][-1:], len(_L3_GROUPS[-1]) - 1)

    nc.compile()
    return nc


def _build_amat_banded(conv_w: np.ndarray) -> np.ndarray:
    """Scatter the 9 conv weights into banded A^T [QP, ABW]."""
    amat = np.zeros((Q, U), np.float32)
    i = np.arange(26)
    j = np.arange(26)
    q = (26 * i[:, None] + j[None, :]).ravel()
    for ki in range(3):
        for kj in range(3):
            u = (28 * (i[:, None] + ki) + j[None, :] + kj).ravel()
            amat[q, u] = conv_w[ki, kj]
    banded = np.zeros((QP, ABW), np.float32)
    for (q0, p_real, c_lo, c_hi) in _fold_bands():
        w = min(U, 128 * c_hi) - 128 * c_lo
        banded[q0:q0 + p_real, :w] = \
            amat[q0:q0 + p_real, 128 * c_lo:128 * c_lo + w]
    return banded


def _make_in_maps(x, conv_w, W1, b1, W2, b2, W3, b3):
    bf = ml_dtypes.bfloat16
    xT = np.ascontiguousarray(np.asarray(x, np.float32).T.astype(bf))
    foldpk = np.zeros((QP, FW), np.float32)
    foldpk[:, :ABW] = _build_amat_banded(np.asarray(conv_w, np.float32))
    foldpk[:Q, ABW:] = np.asarray(W1, np.float32)
    foldpk = np.ascontiguousarray(foldpk.astype(bf))
    W2f = np.asarray(W2, np.float32)
    w2 = np.zeros((3 * KT, H2), np.float32)
    w2[0:H1 - MT] = W2f[0:H1 - MT]
    # k3 chunk: even-block mt halves at parts 0:22 / 32:54, odd-block
    # halves at 64:86 / 96:118 (disjoint, so one shared chunk)
    for base in (0, 64):
        w2[256 + base:256 + base + MTH] = W2f[256:256 + MTH]
        w2[256 + base + 32:256 + base + 32 + MTH] = W2f[256 + MTH:H1]
    w2 = np.ascontiguousarray(w2.astype(bf))
    w3 = np.ascontiguousarray(np.asarray(W3, np.float32).astype(bf))
    bias = np.zeros((128, 5), np.float32)
    b1f = np.asarray(b1, np.float32)
    bias[:, 0] = b1f[0:128]
    bias[:, 1] = b1f[128:256]
    for j in range(4):
        lo = 256 + (j % 2) * MTH
        bias[32 * j:32 * j + MTH, 2] = b1f[lo:lo + MTH]
    bias[:H2, 3] = np.asarray(b2, np.float32)
    b3f = np.asarray(b3, np.float32)
    for j in range(4):
        bias[32 * j:32 * j + H3, 4] = b3f
    in_maps = []
    for c in range(N_CORES):
        in_maps.append({
            "xT": np.ascontiguousarray(xT[:, c * BC:(c + 1) * BC]),
            "fold": foldpk,
            "w2": w2, "w3": w3,
            "bias": bias,
        })
    return in_maps


def kernel(x, conv_w, W1, b1, W2, b2, W3, b3):
    x = np.asarray(x, dtype=np.float32)
    conv_w = np.asarray(conv_w, dtype=np.float32)

    if "nc" not in _prog_cache:
        _prog_cache["nc"] = _build_program()
    nc = _prog_cache["nc"]

    in_maps = _make_in_maps(x, conv_w, W1, b1, W2, b2, W3, b3)
    res = bass_utils.run_bass_kernel_spmd(
        nc, in_maps, core_ids=list(range(N_CORES))
    )

    out = np.empty((B, H3), np.float32)
    for c in range(N_CORES):
        out[c * BC:(c + 1) * BC, :] = res.results[c]["yT"].T
    return out



# revision 4
# speedup vs baseline: 1.1243x; 1.1243x over previous
"""Trainium2 Bass kernel for the digit-conv model, v3 (host fold +
resident x).

Math: y = relu(relu(conv3x3(x) @ W1 + b1) @ W2 + b2) @ W3 + b3.
The valid 3x3 conv folds into W1 on the HOST (W1eff[u] = sum_taps
w_tap * W1[q(u,tap)]), so the device stream is a 3-layer MLP with
channels on partitions and batch on the free dimension.

v3 changes vs v2 (104 us):
  - Conv fold moved to host: no fold DMA, no fold matmuls, no PSUM
    round-trip before the stream starts. Weights ship pre-packed as a
    single [128, 2410] bf16 tensor (w1 k-tiles, padded w1 tail with
    the 0/32/64/96 partition-group replicas, w2 k1k2 + packed k3, w3).
  - All of x stays resident in SBUF (112 KiB/partition): one global
    tile, filled by 12 column-chunk DMAs pre-issued in order on the
    sync queue. No per-super triggers mid-stream, no memsets of the
    tail k-tile (the replica gaps ship as zeros from HBM), so the DMA
    wavefront runs ahead of compute at full bandwidth instead of
    stalling the PE (v2 lost ~5 us to a mid-stream x wait + p-state
    drop).
  - 256-wide blocks on both the ramp AND the tail, so the serial
    epilogue (last pair's L2 -> L3 -> bias -> store) runs at N=256.
  - y stores on the sync HWDGE queue, which is idle after the upfront
    x triggers (gpsimd SWDGE descriptors cost ~640ns each and trailed
    the last matmul by ~3 us).

PE schedule per 512-block (identical tiling to v2):
  - L1 k-tiles of 128 (6 full + K=16 tail), m-chunks {128, 128, 44}.
    The 12 (m0,m1)x(6 k) passes use the full 128x128 array.
  - The K=16 tail for m0+m1 runs as one 32x128-mode row-tiled span
    (even blocks use strips 0,1, odd blocks strips 2,3 so a block
    pair's 4 tail MMs form one span). Tail x rows live pre-replicated
    at partition groups 0/32/64/96 of k-tile 6 (zeros elsewhere).
  - The M=44 chunk (mt) runs col-paired in 128x32 mode: 4 chains
    (2 halves x 2 blocks) at PSUM partition slices 0/32/64/96, each
    contracting 6 full k-tiles plus the zero-padded K=128 tail.
  - L2: two full K=128 passes + k3 (real K=44, zero-padded to 64-row
    strips, an even+odd pair row-tiled into one 64x128-mode span).
  - L3 (K=100, M=10) col-packs 4 blocks into one 128x32-mode span.

All matmul operands bf16 (fp8 DoubleRow fails the 2e-2 gate), PSUM
fp32, biases applied from PSUM in fp32.
"""

import ml_dtypes
import numpy as np

import concourse.tile as tile
from concourse import bacc, mybir
from concourse import bass_utils

N_CORES = 8
B = 65536
BC = B // N_CORES  # 8192 rows per core
U = 784            # input features (28*28)
Q = 676            # conv outputs (26*26)
H1, H2, H3 = 300, 100, 10
NB = 512           # max batch columns per block (one PSUM bank of fp32)
KT = 128           # u-dim k-tile
NKT = 6            # full k-tiles
KTAIL = U - NKT * KT   # 16
MT = 44            # ragged m-chunk width (300 - 2*128)
MTH = 22           # mt half-chunk (one 128x32 col tile per half)
WCOLS = NKT * H1 + H1 + 2 * H2 + H2 + H3  # 2410 packed weight cols

# 256-wide blocks on the ramp (compute starts earlier while x streams
# in) and on the tail (short serial epilogue); full 512 in between.
_BLOCK_NB = [256] * 4 + [512] * 12 + [256] * 4
NBLK = 20
assert sum(_BLOCK_NB) == BC
_BLOCK_START = [0]
for _w in _BLOCK_NB:
    _BLOCK_START.append(_BLOCK_START[-1] + _w)
# x arrives in these column chunks, pre-issued in order on one queue
_XCHUNKS = [256, 256, 512, 512, 512, 1024, 1024, 1024, 1024, 1024, 1024]
assert sum(_XCHUNKS) == BC
_L3_GROUPS = [[0, 1, 2, 3], [4, 5, 6, 7], [8, 9, 10, 11],
              [12, 13, 14, 15], [16, 17], [18, 19]]

_prog_cache = {}


def _build_program():
    f32 = mybir.dt.float32
    bf16 = mybir.dt.bfloat16
    relu = mybir.ActivationFunctionType.Relu
    alu_add = mybir.AluOpType.add
    alu_max = mybir.AluOpType.max

    nc = bacc.Bacc(
        "TRN2", target_bir_lowering=False, debug=False, num_devices=N_CORES
    )

    # x_all: [7*128, BC]: k-tiles 0..5 are xT rows 0..768; k-tile 6 is
    # the K=16 tail pre-replicated at partition groups 0/32/64/96 with
    # zeros in the gaps (so the zero-padded mt tail MM can stream all
    # 128 partitions without any on-device memset).
    x_d = nc.dram_tensor("x", [7 * KT, BC], bf16, kind="ExternalInput").ap()
    wts_d = nc.dram_tensor("wts", [128, WCOLS], bf16, kind="ExternalInput").ap()
    bias_d = nc.dram_tensor("bias", [128, 5], f32, kind="ExternalInput").ap()
    yT_d = nc.dram_tensor("yT", [H3, BC], f32, kind="ExternalOutput").ap()

    with tile.TileContext(nc) as tc:
        with tc.tile_pool(name="const", bufs=1) as cpool, \
             tc.tile_pool(name="hp", bufs=4) as hpool, \
             tc.tile_pool(name="h2p", bufs=5) as h2pool, \
             tc.tile_pool(name="yp", bufs=2) as ypool, \
             tc.tile_pool(name="ps1", bufs=5, space="PSUM") as ps1p, \
             tc.tile_pool(name="psmt", bufs=1, space="PSUM") as psmtp, \
             tc.tile_pool(name="ps2", bufs=2, space="PSUM") as ps2p:

            # ---- weights + x DMAs, pre-issued in order on sync ----
            wts_sb = cpool.tile([128, WCOLS], bf16)
            nc.sync.dma_start(wts_sb[:], wts_d)
            bias_sb = cpool.tile([128, 5], f32)
            nc.scalar.dma_start(bias_sb[:], bias_d)

            xt = cpool.tile([128, 7 * BC], bf16)
            xv = xt[:].rearrange("p (k c) -> p k c", c=BC)
            xsrc = x_d.rearrange("(k p) c -> p k c", p=128)
            c0 = 0
            for w in _XCHUNKS:
                nc.sync.dma_start(xv[:, :, c0:c0 + w], xsrc[:, :, c0:c0 + w])
                c0 += w

            # ---- HAM / p-state warmup while the first DMAs land ----
            warm_sb = cpool.tile([128, 512], bf16)
            nc.vector.memset(warm_sb[:], 0.0)
            for wi in range(8):
                pw = psmtp.tile([128, NB], f32, tag="mt", name=f"pwarm_{wi}")
                nc.tensor.matmul(pw[:], warm_sb[:, :128], warm_sb[:],
                                 start=True, stop=True)

            # ---- weight views ----
            w1v = wts_sb[:, 0:NKT * H1].rearrange("p (k c) -> p k c", c=H1)
            w1t = wts_sb[:, NKT * H1:NKT * H1 + H1]       # packed tail
            w2o = NKT * H1 + H1
            w2v = wts_sb[:, w2o:w2o + 3 * H2].rearrange("p (k c) -> p k c",
                                                        c=H2)
            w3_sb = wts_sb[:, w2o + 3 * H2:w2o + 3 * H2 + H3]

            psA, psB, h1t, h2t, p2t = {}, {}, {}, {}, {}

            def blk(b):
                return _BLOCK_START[b], _BLOCK_NB[b]

            def alloc_h1(b):
                if b in h1t:
                    return
                h1 = hpool.tile([128, 3 * NB], bf16, tag="h1", name=f"h1_{b}")
                h1t[b] = h1
                h1v = h1[:].rearrange("p (k c) -> p k c", c=NB)
                # parts outside the mt halves in the third chunk must be
                # zero for the zero-padded L2 k3 pass; cleared per block
                # so every logical tile has the region written (the mt
                # relus overwrite their slices).
                nc.vector.memset(h1v[:, 2, :], 0.0)

            def s1_block(b):
                # k-tail span: m0+m1 tails row-tiled (opens the psum groups)
                off, nb = blk(b)
                pA = ps1p.tile([128, NB], f32, tag="l1", name=f"pA_{b}")
                pB = ps1p.tile([128, NB], f32, tag="l1", name=f"pB_{b}")
                psA[b], psB[b] = pA, pB
                alloc_h1(b)
                p0 = 0 if b % 2 == 0 else 64
                p1 = p0 + 32
                nc.tensor.matmul(pA[:, :nb], w1t[p0:p0 + KTAIL, 0:128],
                                 xv[p0:p0 + KTAIL, 6, off:off + nb],
                                 start=True, stop=False)
                kw = {"tile_position": (96, 0)} if p1 == 96 else {}
                nc.tensor.matmul(pB[:, :nb], w1t[p1:p1 + KTAIL, 128:256],
                                 xv[p1:p1 + KTAIL, 6, off:off + nb],
                                 start=True, stop=False, **kw)

            def s2_block(b):
                # 12 full 128x128 passes + relu drains for m0/m1
                off, nb = blk(b)
                pA, pB = psA[b], psB[b]
                h1v = h1t[b][:].rearrange("p (k c) -> p k c", c=NB)
                for kt in range(NKT):
                    nc.tensor.matmul(pA[:, :nb], w1v[:, kt, 0:128],
                                     xv[:, kt, off:off + nb],
                                     start=False, stop=(kt == NKT - 1))
                nc.scalar.activation(h1v[:, 0, :nb], pA[:, :nb], relu,
                                     bias=bias_sb[:, 0:1], scale=1.0)
                for kt in range(NKT):
                    nc.tensor.matmul(pB[:, :nb], w1v[:, kt, 128:256],
                                     xv[:, kt, off:off + nb],
                                     start=False, stop=(kt == NKT - 1))
                nc.scalar.activation(h1v[:, 1, :nb], pB[:, :nb], relu,
                                     bias=bias_sb[:, 1:2], scale=1.0)

            def s3_pair(b0, b1):
                # mt (44 channels) split 22+22 across col tiles: 4 chains
                # (2 halves x 2 blocks) run concurrently in 128x32 mode at
                # PSUM partition slices 0:22 / 32:54 / 64:86 / 96:118, each
                # contracting all 7 k-tiles. The four chains are independent
                # per-partition accumulation groups in one bank; the bass
                # group checker only models bank-granular groups, so skip it.
                pm = psmtp.tile([128, NB], f32, tag="mt", name=f"pmt_{b0}")
                alloc_h1(b0)
                alloc_h1(b1)
                chains = []  # (psum base, block, w1 col lo/hi)
                for j, b in enumerate((b0, b0, b1, b1)):
                    lo = 256 + (j % 2) * MTH
                    chains.append((32 * j, b, lo, lo + MTH))
                for kt in range(NKT + 1):
                    for (pb_, b, lo, hi) in chains:
                        off, nb = blk(b)
                        if kt < NKT:
                            lhs = w1v[:, kt, lo:hi]
                        else:
                            lhs = w1t[:, lo:hi]
                        nc.tensor.matmul(pm[pb_:pb_ + MTH, :nb], lhs,
                                         xv[:, min(kt, 6), off:off + nb],
                                         start=(kt == 0), stop=(kt == NKT),
                                         skip_group_check=True,
                                         tile_position=(0, pb_))
                for (pb_, b, lo, hi) in chains:
                    nb = _BLOCK_NB[b]
                    h1v = h1t[b][:].rearrange("p (k c) -> p k c", c=NB)
                    nc.scalar.activation(
                        h1v[pb_:pb_ + MTH, 2, :nb], pm[pb_:pb_ + MTH, :nb],
                        relu, bias=bias_sb[pb_:pb_ + MTH, 2:3], scale=1.0)

            def l2_head(b):
                # the two full K=128 passes of L2 (group opened here)
                nb = _BLOCK_NB[b]
                h1v = h1t[b][:].rearrange("p (k c) -> p k c", c=NB)
                p2 = ps2p.tile([H2, NB], f32, tag="l2", name=f"p2_{b}",
                               padded_shape=[128, NB])
                p2t[b] = p2
                for j in range(2):
                    nc.tensor.matmul(p2[:, :nb], w2v[:, j, :],
                                     h1v[:, j, :nb],
                                     start=(j == 0), stop=False)

            def l2_tail_pair(b0, b1):
                # k3 (real K=44, zero-padded): the mt halves of even blocks
                # live at parts 0:54, odd at 64:118, so an even+odd pair of
                # k3 passes row-tiles into one 64x128-mode span (different
                # psum banks), then both h2 relus drain.
                for i, b in enumerate((b0, b1)):
                    base = 64 * (b % 2)
                    nb = _BLOCK_NB[b]
                    h1v = h1t[b][:].rearrange("p (k c) -> p k c", c=NB)
                    nc.tensor.matmul(p2t[b][:, :nb],
                                     w2v[base:base + 64, 2, :],
                                     h1v[base:base + 64, 2, :nb],
                                     start=False, stop=True,
                                     tile_position=(base, 0))
                for b in (b0, b1):
                    nb = _BLOCK_NB[b]
                    h2 = h2pool.tile([H2, NB], bf16, tag="h2", name=f"h2_{b}")
                    h2t[b] = h2
                    nc.vector.tensor_scalar(h2[:, :nb], p2t[b][:, :nb],
                                            bias_sb[0:H2, 3:4], 0.0,
                                            alu_add, alu_max)

            def l3_span(bs, j0):
                # blocks col-packed in 128x32 mode, then bias + store
                p3 = ps2p.tile([128, NB], f32, tag="l2", name=f"p3_{bs[0]}")
                y = ypool.tile([128, NB], f32, tag="y", name=f"y_{bs[0]}")
                for i, b in enumerate(bs):
                    j = j0 + i
                    nb = _BLOCK_NB[b]
                    nc.tensor.matmul(p3[32 * j:32 * j + H3, :nb],
                                     w3_sb[0:H2, :],
                                     h2t[b][:, :nb], start=True, stop=True,
                                     tile_position=(0, 32 * j))
                for i, b in enumerate(bs):
                    j = j0 + i
                    c0, nb = blk(b)
                    nc.vector.tensor_scalar_add(
                        y[32 * j:32 * j + H3, :nb],
                        p3[32 * j:32 * j + H3, :nb],
                        bias_sb[32 * j:32 * j + H3, 4:5])
                    nc.sync.dma_start(yT_d[:, c0:c0 + nb],
                                      y[32 * j:32 * j + H3, :nb])

            next_g = [0]

            def fire_l3(upto_block):
                while (next_g[0] < len(_L3_GROUPS) - 1
                       and _L3_GROUPS[next_g[0]][-1] <= upto_block):
                    l3_span(_L3_GROUPS[next_g[0]], 0)
                    next_g[0] += 1

            for p in range(NBLK // 2):
                b0, b1 = 2 * p, 2 * p + 1
                last = p == NBLK // 2 - 1
                s1_block(b0)
                s1_block(b1)
                if last:
                    # last pair: mt before the m-chains so its relus are
                    # long done when the trailing L2/L3 chain reads them.
                    s3_pair(b0, b1)
                s2_block(b0)
                s2_block(b1)
                # L2 of the previous pair goes after this pair's m-chains so
                # the relus that feed it (end of previous pair) are long
                # done by the time its passes read h1.
                if p >= 1:
                    l2_head(b0 - 2)
                    l2_head(b1 - 2)
                    l2_tail_pair(b0 - 2, b1 - 2)
                if not last:
                    s3_pair(b0, b1)
                if p >= 1:
                    fire_l3(b1 - 2)
            # epilogue: the last pair's L2s, then the final L3 group in two
            # spans so only one block's bias-add + store trails the stream.
            l2_head(NBLK - 2)
            l2_head(NBLK - 1)
            l2_tail_pair(NBLK - 2, NBLK - 1)
            l3_span(_L3_GROUPS[-1][:-1], 0)
            l3_span(_L3_GROUPS[-1][-1:], len(_L3_GROUPS[-1]) - 1)

    nc.compile()
    return nc


def _fold_w1_host(conv_w: np.ndarray, W1: np.ndarray) -> np.ndarray:
    """W1eff [U, H1]: W1eff[u] = sum_taps conv_w[ki,kj] * W1[q(u,ki,kj)]."""
    w1eff = np.zeros((U, H1), np.float32)
    i = np.arange(26)
    j = np.arange(26)
    for ki in range(3):
        for kj in range(3):
            u = (28 * (i[:, None] + ki) + j[None, :] + kj).ravel()
            w1eff[u] += conv_w[ki, kj] * W1
    return w1eff


def _make_in_maps(x, conv_w, W1, b1, W2, b2, W3, b3):
    bf = ml_dtypes.bfloat16
    xT = np.asarray(x, np.float32).T.astype(bf)  # [U, B]

    w1eff = _fold_w1_host(np.asarray(conv_w, np.float32),
                          np.asarray(W1, np.float32))
    # packed tail [128, 300]: K=16 tail weights replicated at partition
    # groups for the row-tiled S1 span (m0 at 0/64 cols 0:128, m1 at
    # 32/96 cols 128:256) and at parts 0:16 for the mt chains' cols
    # 256:300; zeros elsewhere so the zero-padded mt tail MM is exact.
    tail = w1eff[NKT * KT:U]  # [16, 300]
    w1t = np.zeros((128, H1), np.float32)
    w1t[0:KTAIL] = tail
    w1t[32:32 + KTAIL, 128:256] = tail[:, 128:256]
    w1t[64:64 + KTAIL, 0:128] = tail[:, 0:128]
    w1t[96:96 + KTAIL, 128:256] = tail[:, 128:256]

    W2f = np.asarray(W2, np.float32)
    # k3 chunk: even-block mt halves at parts 0:22 / 32:54, odd-block
    # halves at 64:86 / 96:118 (disjoint, so one shared chunk)
    w2k3 = np.zeros((128, H2), np.float32)
    for base in (0, 64):
        w2k3[base:base + MTH] = W2f[256:256 + MTH]
        w2k3[base + 32:base + 32 + MTH] = W2f[256 + MTH:H1]

    w3p = np.zeros((128, H3), np.float32)
    w3p[0:H2] = np.asarray(W3, np.float32)

    # single packed weight tensor [128, WCOLS]
    wts = np.zeros((128, WCOLS), np.float32)
    for kt in range(NKT):
        wts[:, kt * H1:(kt + 1) * H1] = w1eff[kt * KT:(kt + 1) * KT]
    wts[:, NKT * H1:NKT * H1 + H1] = w1t
    w2o = NKT * H1 + H1
    wts[:, w2o:w2o + H2] = W2f[0:128]
    wts[:, w2o + H2:w2o + 2 * H2] = W2f[128:256]
    wts[:, w2o + 2 * H2:w2o + 3 * H2] = w2k3
    wts[:, w2o + 3 * H2:w2o + 3 * H2 + H3] = w3p
    wts = np.ascontiguousarray(wts.astype(bf))

    bias = np.zeros((128, 5), np.float32)
    b1f = np.asarray(b1, np.float32)
    bias[:, 0] = b1f[0:128]
    bias[:, 1] = b1f[128:256]
    for j in range(4):
        lo = 256 + (j % 2) * MTH
        bias[32 * j:32 * j + MTH, 2] = b1f[lo:lo + MTH]
    bias[:H2, 3] = np.asarray(b2, np.float32)
    b3f = np.asarray(b3, np.float32)
    for j in range(4):
        bias[32 * j:32 * j + H3, 4] = b3f

    in_maps = []
    for c in range(N_CORES):
        xc = xT[:, c * BC:(c + 1) * BC]
        # x_all [7*128, BC]: 6 main k-tiles + pre-replicated padded tail
        xa = np.zeros((7 * KT, BC), bf)
        xa[0:NKT * KT] = xc[0:NKT * KT]
        for g in range(4):
            xa[NKT * KT + 32 * g:NKT * KT + 32 * g + KTAIL] = xc[NKT * KT:U]
        in_maps.append({
            "x": np.ascontiguousarray(xa),
            "wts": wts,
            "bias": bias,
        })
    return in_maps


def kernel(x, conv_w, W1, b1, W2, b2, W3, b3):
    x = np.asarray(x, dtype=np.float32)
    conv_w = np.asarray(conv_w, dtype=np.float32)

    if "nc" not in _prog_cache:
        _prog_cache["nc"] = _build_program()
    nc = _prog_cache["nc"]

    in_maps = _make_in_maps(x, conv_w, W1, b1, W2, b2, W3, b3)
    res = bass_utils.run_bass_kernel_spmd(
        nc, in_maps, core_ids=list(range(N_CORES))
    )

    out = np.empty((B, H3), np.float32)
    for c in range(N_CORES):
        out[c * BC:(c + 1) * BC, :] = res.results[c]["yT"].T
    return out
